# revision 25
# baseline (speedup 1.0000x reference)
"""Trainium2 Bass kernel for nn_AttentionAggregator (GNN message passing).

Math (per batch row b, N=64 neighbors, F=128 in-feat, H=8 heads, D=64):
    lin  = x @ W_lin                                      [B, N, 512]
    att  = lin[:,0,:] @ W_att[:512] + lin @ W_att[512:]   [B, N, 8]
    att  = LeakyReLU_0.2(att); masked softmax over N per (b, h)
    out  = relu(lin) * aw                                 [B, N, 512]

Design (v4, row-major attention): the original design computed attention
head-major and burned ~100us/core of PE on head-expander/transpose matmuls
plus 512-col triplet streams. v4 computes att ROW-major on the PE as one
tiny 8-col matmul per 128-row chunk (att[row, :] = x_chunk.T @ wc2, with
wc2 = W_lin @ W_att[512:]). The src term (x[b,0] @ wc1, 8 MFLOP = 0.02% of
model FLOPs) plus the {0,-6e4} mask bias are packed on the host into a
per-row f16 bias slab, added by one DVE op per mega before the LeakyReLU
-- both are legal pre-softmax because LeakyReLU(x - big) still exps to 0
and the src add commutes with nothing else in between.

Rows are host-permuted within each 512-row pair so row = 4*p + k (k =
chunk 0..3, p = partition); a batch element's 64 rows then live in a
16-partition block across 4 chunk-columns, so the softmax denominator is
one block-ones16 matmul (partition sum, broadcast for free) + a DVE
k-reduce + reciprocal. aw comes out row-major [p, (j k h)] -- no
transposes or expanders at all.

Final multiply per 256-row half-pair, two balanced paths:
  A (DVE):      stt out = max(lin_f32psum,0) * aw_bcast        (~1.45us)
  B (ACT+DVE):  ACT relu lin->rlin f16 sbuf                    (~1.3us)
                one DVE tt rlin * awr4 view                    (~0.65us)
awr4 is aw replicated 4x along a packed last dim (one ACT broadcast-copy
per mega): viewing d = 16*dA + dB with dA stride-0 and dB stride-1 keeps
enough 16-bit packing for the DVE 2x mode, unlike a plain stride-0
broadcast which drops DVE to 1x (measured).

The row permutation also makes each output-DMA descriptor 4KB contiguous
(rows 4p..4p+3 per partition), and x loads are one 8KB/partition DMA per
16-tile mega. Software pipeline: fronts(m+2) att matmuls and chain(m+1)
(add/Prelu/Exp/den/recip/aw/awr4) are threaded into backs(m) at
dependency-ready points. PSUM start_tensor_calc marks a whole 2KB zero
region pending: exactly one start per mega per bank.

Sharding: pure data-parallel over batch: 512 batch rows per core,
weights replicated. fp16 I/O; host packs/upcasts.
"""

import os
from contextlib import ExitStack

import numpy as np

import concourse.bacc as bacc
import concourse.bass as bass
import concourse.tile as tile
from concourse import mybir
from concourse.bass_utils import run_bass_kernel_spmd

B, N, F = 4096, 64, 128
H, D = 8, 64
HD = H * D  # 512
NCORES = 8
BSHARD = B // NCORES  # 512
ROWS = BSHARD * N  # 32768
PAIR_ROWS = 512  # rows per pair (4 chunks of 128)
NPAIRS = ROWS // PAIR_ROWS  # 64
MPAIRS = 8  # pairs per mega
NMEGA = NPAIRS // MPAIRS  # 8
# half-pair slots (0..15) on pipeline A (DVE stt); rest pipeline B
A_SLOTS = (0, 3, 6, 9, 12)

f32 = mybir.dt.float32
f16 = mybir.dt.float16

LAST_RESULT = None  # test harness reads exec_time_ns / trace from here


def build_nc(nmega: int = NMEGA) -> bass.Bass:
    nc = bacc.Bacc("TRN2", target_bir_lowering=False, debug=False)
    rows = nmega * MPAIRS * PAIR_ROWS

    xt = nc.declare_dram_parameter("xt", [nmega, F, MPAIRS * PAIR_ROWS], f16,
                                   isOutput=False)
    wlin_d = nc.declare_dram_parameter("wlin", [F, HD], f16, isOutput=False)
    wc2_d = nc.declare_dram_parameter("wc2", [F, 8], f16, isOutput=False)
    bones_d = nc.declare_dram_parameter("bones", [128, 128], f16, isOutput=False)
    attm_d = nc.declare_dram_parameter("attm", [nmega, 128, 256], f16,
                                       isOutput=False)
    out = nc.declare_dram_parameter("out", [rows, HD], f16, isOutput=True)

    mult = mybir.AluOpType.mult
    mmax = mybir.AluOpType.max
    madd = mybir.AluOpType.add

    with tile.TileContext(nc) as tc, ExitStack() as ctx:
        consts = ctx.enter_context(tc.tile_pool(name="consts", bufs=1))
        xin = ctx.enter_context(tc.tile_pool(name="xin", bufs=6))
        amp = ctx.enter_context(tc.tile_pool(name="amp", bufs=3))
        rlinp = ctx.enter_context(tc.tile_pool(name="rlinp", bufs=4))
        outp = ctx.enter_context(tc.tile_pool(name="outp", bufs=4))
        plin = ctx.enter_context(tc.tile_pool(name="plin", bufs=3, space="PSUM"))
        pchain = ctx.enter_context(tc.tile_pool(name="pchain", bufs=1, space="PSUM"))

        # wc2 first: fronts(0) needs it; wlin/bones DMAs issue after loads(0)
        wc2_sb = consts.tile([F, 8], f16)
        nc.sync.dma_start(out=wc2_sb, in_=wc2_d[:])
        wlin_sb = consts.tile([F, HD], f16)
        bones_sb = consts.tile([128, 128], f16)
        alpha_sb = consts.tile([128, 1], f32)
        nc.vector.memset(alpha_sb, 0.2)
        ebias_sb = consts.tile([128, 1], f32)
        nc.vector.memset(ebias_sb, -4.0)

        # per-mega chain slabs, ping-pong x2. free-dim order is (j, k, h):
        # j pair 0..7 (stride 32), k chunk (stride 8), h head (stride 1)
        def mk_slabs(i):
            asum = consts.tile([128, 256], f16, tag=f"asum{i}")
            lk = consts.tile([128, 256], f16, tag=f"lk{i}")
            ew = consts.tile([128, 256], f16, tag=f"ew{i}")
            den4 = consts.tile([128, MPAIRS, H, 1], f32, tag=f"den4{i}")
            rden = consts.tile([128, MPAIRS, H, 1], f16, tag=f"rden{i}")
            aw = consts.tile([128, 256], f16, tag=f"aw{i}")
            awr4 = consts.tile([128, 256, 4], f16, tag=f"awr4{i}")
            return {"asum": asum, "lk": lk, "ew": ew, "den4": den4,
                    "rden": rden, "aw": aw, "awr4": awr4}

        slabs = [mk_slabs(0), mk_slabs(1)]
        # att accumulation + den, ping-pong PSUM: [:, 0, :] att, [:, 1, :] den
        att_ps = [pchain.tile([128, 2, 256], f32, tag=f"attps{i}",
                              name=f"attps{i}")
                  for i in range(2)]

        def loads(m):
            # two half-mega x DMAs so fronts can start after the first lands
            xa = xin.tile([F, 4, 4, 128], f16, tag="xa", name="xa")
            nc.sync.dma_start(
                out=xa.rearrange("f j k p -> f (j k p)"),
                in_=xt[m][:, 0:4 * PAIR_ROWS])
            xb = xin.tile([F, 4, 4, 128], f16, tag="xb", name="xb")
            nc.sync.dma_start(
                out=xb.rearrange("f j k p -> f (j k p)"),
                in_=xt[m][:, 4 * PAIR_ROWS:])
            am_sb = amp.tile([128, 256], f16, tag="am")
            nc.sync.dma_start(out=am_sb, in_=attm_d[m])

            def xv(j, k):
                return (xa if j < 4 else xb)[:, j % 4, k, :]

            return {"xv": xv, "am": am_sb}

        def fronts_pair(m, j, ld):
            # att[p, (j k h)] accumulates in PSUM. start_tensor_calc marks the
            # whole 2KB zero region pending: issue it ONLY on the mega's first
            # matmul; afterwards first touch of each address overwrites.
            av = att_ps[m % 2][:, 0, :].rearrange(
                "p (j k h) -> p j k h", j=MPAIRS, k=4, h=H)
            for k in range(4):
                nc.tensor.matmul(
                    av[:, j, k, :], ld["xv"](j, k), wc2_sb,
                    start=(j == 0 and k == 0),
                    stop=(j == MPAIRS - 1 and k == 3),
                    skip_group_check=True)

        def chain_add(m, ld):
            s = slabs[m % 2]
            nc.vector.tensor_tensor(out=s["asum"], in0=att_ps[m % 2][:, 0, :],
                                    in1=ld["am"], op=madd)

        def chain_act(m):
            s = slabs[m % 2]
            nc.scalar.activation(
                out=s["lk"], in_=s["asum"],
                func=mybir.ActivationFunctionType.Prelu, alpha=alpha_sb)
            nc.scalar.activation(
                out=s["ew"], in_=s["lk"],
                func=mybir.ActivationFunctionType.Exp, bias=ebias_sb)

        def chain_den(m):
            # den region is pending-zero from the mega's fronts start flag:
            # start=False means this first write overwrites, not accumulates
            s = slabs[m % 2]
            nc.tensor.matmul(att_ps[m % 2][:, 1, :], bones_sb, s["ew"],
                             start=False, stop=True, skip_group_check=True)

        def chain_dve(m):
            s = slabs[m % 2]
            dv = att_ps[m % 2][:, 1, :].rearrange(
                "p (j k h) -> p j h k", j=MPAIRS, k=4, h=H)
            nc.vector.tensor_reduce(out=s["den4"], in_=dv,
                                    axis=mybir.AxisListType.X, op=madd)
            with nc.allow_low_precision(reason="aw weights are f16 anyway"):
                nc.vector.reciprocal(s["rden"], s["den4"])

        def chain_awt(m):
            s = slabs[m % 2]
            awv = s["aw"].rearrange("p (j k h) -> p j k h", j=MPAIRS, k=4, h=H)
            ewv = s["ew"].rearrange("p (j k h) -> p j k h", j=MPAIRS, k=4, h=H)
            rdv = s["rden"].rearrange("p j h one -> p j (h one)")
            for k in range(4):
                nc.gpsimd.tensor_tensor(
                    out=awv[:, :, k, :], in0=ewv[:, :, k, :], in1=rdv, op=mult)

        def chain_awr4(m):
            s = slabs[m % 2]
            nc.scalar.copy(out=s["awr4"],
                           in_=s["aw"].to_broadcast([128, 256, 4]))

        def backs_half(m, j, u, ld, state):
            s = slabs[m % 2]
            slot = j * 2 + u
            if u == 0:
                state["o2"] = outp.tile([128, 4, HD], f16, tag="o2",
                                        name="o2")
            o2 = state["o2"]
            lin_ps = plin.tile([128, 2, HD], f32, tag="lin")
            for c in (0, 1):
                nc.tensor.matmul(lin_ps[:, c, :], ld["xv"](j, 2 * u + c),
                                 wlin_sb, start=True, stop=True)
            bb_pair = (2 * j not in A_SLOTS) and (2 * j + 1 not in A_SLOTS)
            if slot in A_SLOTS:
                aw16 = s["aw"][:, j * 32 + 16 * u:j * 32 + 16 * u + 16]
                nc.vector.scalar_tensor_tensor(
                    out=o2[:, 2 * u:2 * u + 2, :].rearrange(
                        "p k (h d) -> p (k h) d", h=H),
                    in0=lin_ps.rearrange("p k (h d) -> p (k h) d", h=H),
                    scalar=0.0,
                    in1=aw16.to_broadcast([128, 2 * H, D]),
                    op0=mmax, op1=mult)
            elif bb_pair:
                # both halves B: one merged [128, 2048] tt for the whole pair
                if u == 0:
                    state["rlbb"] = rlinp.tile([128, 4, HD], f16, tag="rlbb",
                                               name="rlbb")
                rlin4 = state["rlbb"]
                nc.scalar.activation(
                    out=rlin4[:, 2 * u:2 * u + 2, :].rearrange(
                        "p a b -> p (a b)"),
                    in_=lin_ps.rearrange("p a b -> p (a b)"),
                    func=mybir.ActivationFunctionType.Relu)
                if u == 1:
                    state.setdefault("pend", []).append(
                        ("BB", m, j, 0, rlin4, o2))
            else:
                rlin = rlinp.tile([128, 2, HD], f16, tag="rlin")
                nc.scalar.activation(
                    out=rlin.rearrange("p a b -> p (a b)"),
                    in_=lin_ps.rearrange("p a b -> p (a b)"),
                    func=mybir.ActivationFunctionType.Relu)
                # defer the DVE tt one half-pair so ACT has lead time
                state.setdefault("pend", []).append(("B", m, j, u, rlin, o2))
            if u == 1:
                state.setdefault("dma", []).append((j, o2))

        def flush_tts(state, keep=1):
            s_pend = state.get("pend", [])
            while len(s_pend) > keep:
                kind, m, j, u, rlin, o2 = s_pend.pop(0)
                s = slabs[m % 2]
                a4v = s["awr4"].rearrange(
                    "p (j kh) r -> p j kh r", j=MPAIRS, kh=32)
                if kind == "BB":
                    in1 = a4v[:, j, :, :].rearrange(
                        "p kh (one r) -> p kh one r", one=1
                    ).broadcast_to([128, 32, 16, 4])
                    nc.vector.tensor_tensor(
                        out=o2.rearrange(
                            "p k (h da db) -> p (k h) da db", h=H, da=16),
                        in0=rlin.rearrange(
                            "p k (h da db) -> p (k h) da db", h=H, da=16),
                        in1=in1, op=mult)
                else:
                    in1 = a4v[:, j, 16 * u:16 * u + 16, :].rearrange(
                        "p kh (one r) -> p kh one r", one=1
                    ).broadcast_to([128, 16, 16, 4])
                    nc.vector.tensor_tensor(
                        out=o2[:, 2 * u:2 * u + 2, :].rearrange(
                            "p k (h da db) -> p (k h) da db", h=H, da=16),
                        in0=rlin.rearrange(
                            "p k (h da db) -> p (k h) da db", h=H, da=16),
                        in1=in1, op=mult)

        def flush_dma(m, state, keep=1):
            dmas = state.get("dma", [])
            while len(dmas) > keep:
                j, o2 = dmas.pop(0)
                base = (m * MPAIRS + j) * PAIR_ROWS
                out_view = out[base:base + PAIR_ROWS, :].rearrange(
                    "(p four) hd -> p (four hd)", four=4)
                nc.sync.dma_start(
                    out=out_view, in_=o2.rearrange("p k hd -> p (k hd)"))

        # ---- software pipeline ----
        ld = {0: loads(0)}
        nc.sync.dma_start(out=wlin_sb, in_=wlin_d[:])
        nc.sync.dma_start(out=bones_sb, in_=bones_d[:])
        for j in range(MPAIRS):
            fronts_pair(0, j, ld[0])
        chain_add(0, ld[0])
        chain_act(0)
        chain_den(0)
        chain_dve(0)
        chain_awt(0)
        chain_awr4(0)
        if nmega > 1:
            ld[1] = loads(1)
            for j in range(MPAIRS):
                fronts_pair(1, j, ld[1])

        for m in range(nmega):
            state = {}
            for j in range(MPAIRS):
                for u in (0, 1):
                    backs_half(m, j, u, ld[m], state)
                    flush_tts(state, keep=1)
                flush_dma(m, state, keep=1)
                if j == 0 and m + 2 < nmega:
                    ld[m + 2] = loads(m + 2)
                if m + 1 < nmega:
                    if j == 1:
                        chain_add(m + 1, ld[m + 1])
                    elif j == 2:
                        chain_act(m + 1)
                    elif j == 3:
                        chain_den(m + 1)
                    elif j == 4:
                        chain_dve(m + 1)
                    elif j == 5:
                        chain_awt(m + 1)
                    elif j == 6:
                        chain_awr4(m + 1)
                if m + 2 < nmega and j >= 4:
                    fronts_pair(m + 2, 2 * (j - 4), ld[m + 2])
                    fronts_pair(m + 2, 2 * (j - 4) + 1, ld[m + 2])
            flush_tts(state, keep=0)
            flush_dma(m, state, keep=0)
            ld.pop(m, None)

    nc.compile()
    return nc


def _host_weights(W_lin, W_att):
    W_lin64 = W_lin.astype(np.float64)
    wc2 = (W_lin64 @ W_att[HD:].astype(np.float64)).astype(np.float16)
    wc1 = (W_lin64 @ W_att[:HD].astype(np.float64)).astype(np.float32)
    bones = np.zeros((128, 128), dtype=np.float16)
    for b in range(8):
        bones[16 * b:16 * b + 16, 16 * b:16 * b + 16] = 1.0
    return W_lin.astype(np.float16), wc2, wc1, bones


def _core_inputs(x_shard, mask_shard, wlin, wc2, wc1):
    nb = x_shard.shape[0]
    rows = nb * N
    nmega = rows // (MPAIRS * PAIR_ROWS)
    xsh = x_shard.reshape(rows, F)
    # x_pack[m, f, (j, k, p)] = x[row = m*4096 + j*512 + 4p + k, f]
    xp = (xsh.reshape(nmega, MPAIRS, 128, 4, F)
          .transpose(0, 4, 1, 3, 2)
          .reshape(nmega, F, MPAIRS * PAIR_ROWS)
          .astype(np.float16))
    # attm[m, p, (j, k, h)] = attsrc[batch(j, p//16), h] + {0, -6e4}[mask]
    attsrc = x_shard[:, 0, :].astype(np.float32) @ wc1  # [nb, H]
    a1 = attsrc.reshape(nmega, MPAIRS, 8, H)
    a2 = np.repeat(a1, 16, axis=2)                      # m, j, p, h
    a3 = a2.transpose(0, 2, 1, 3)                       # m, p, j, h
    a4 = np.broadcast_to(a3[:, :, :, None, :], (nmega, 128, MPAIRS, 4, H))
    mb = np.where(mask_shard.reshape(rows) != 0, 0.0, -60000.0).astype(
        np.float32)
    mv = mb.reshape(nmega, MPAIRS, 128, 4).transpose(0, 2, 1, 3)  # m,p,j,k
    attm = (a4 + mv[..., None]).astype(np.float16).reshape(nmega, 128, 256)
    return {
        "xt": np.ascontiguousarray(xp),
        "wlin": wlin,
        "wc2": wc2,
        "bones": np.zeros((0,)),  # replaced below
        "attm": np.ascontiguousarray(attm),
    }


def kernel(x, W_lin, W_att, mask):
    global LAST_RESULT
    x = np.asarray(x, dtype=np.float32)
    W_lin = np.asarray(W_lin, dtype=np.float32)
    W_att = np.asarray(W_att, dtype=np.float32)
    mask = np.asarray(mask)

    wlin, wc2, wc1, bones = _host_weights(W_lin, W_att)
    in_maps = []
    for c in range(NCORES):
        im = _core_inputs(
            x[c * BSHARD:(c + 1) * BSHARD],
            mask[c * BSHARD:(c + 1) * BSHARD],
            wlin, wc2, wc1,
        )
        im["bones"] = bones
        in_maps.append(im)

    nc = build_nc(NMEGA)
    trace = os.environ.get("KERNEL_TRACE", "0") == "1"
    tmpdir = os.environ.get("KERNEL_TRACE_DIR") or None
    res = run_bass_kernel_spmd(
        nc, in_maps, list(range(NCORES)), trace=trace, tmpdir=tmpdir
    )
    LAST_RESULT = res
    return np.concatenate(
        [
            res.results[c]["out"].astype(np.float32).reshape(BSHARD, N, HD)
            for c in range(NCORES)
        ],
        axis=0,
    )


# revision 26
# speedup vs baseline: 1.2039x; 1.2039x over previous
"""Trainium2 Bass kernel for nn_AttentionAggregator (GNN message passing).

Math (per batch row b, N=64 neighbors, F=128 in-feat, H=8 heads, D=64):
    lin  = x @ W_lin                                      [B, N, 512]
    att  = lin[:,0,:] @ W_att[:512] + lin @ W_att[512:]   [B, N, 8]
    att  = LeakyReLU_0.2(att); masked softmax over N per (b, h)
    out  = relu(lin) * aw                                 [B, N, 512]

Design (v4, row-major attention): the original design computed attention
head-major and burned ~100us/core of PE on head-expander/transpose matmuls
plus 512-col triplet streams. v4 computes att ROW-major on the PE as one
tiny 8-col matmul per 128-row chunk (att[row, :] = x_chunk.T @ wc2, with
wc2 = W_lin @ W_att[512:]). The src term (x[b,0] @ wc1, 8 MFLOP = 0.02% of
model FLOPs) plus the {0,-6e4} mask bias are packed on the host into a
per-row f16 bias slab, added by one DVE op per mega before the LeakyReLU
-- both are legal pre-softmax because LeakyReLU(x - big) still exps to 0
and the src add commutes with nothing else in between.

Rows are host-permuted within each 512-row pair so row = 4*p + k (k =
chunk 0..3, p = partition); a batch element's 64 rows then live in a
16-partition block across 4 chunk-columns, so the softmax denominator is
one block-ones16 matmul (partition sum, broadcast for free) + a DVE
k-reduce + reciprocal. aw comes out row-major [p, (j k h)] -- no
transposes or expanders at all.

Final multiply per 256-row half-pair, two balanced paths:
  A (DVE):      stt out = max(lin_f32psum,0) * aw_bcast        (~1.45us)
  B (ACT+DVE):  ACT relu lin->rlin f16 sbuf                    (~1.3us)
                one DVE tt rlin * awr4 view                    (~0.65us)
awr4 is aw replicated 4x along a packed last dim (one ACT broadcast-copy
per mega): viewing d = 16*dA + dB with dA stride-0 and dB stride-1 keeps
enough 16-bit packing for the DVE 2x mode, unlike a plain stride-0
broadcast which drops DVE to 1x (measured).

The row permutation also makes each output-DMA descriptor 4KB contiguous
(rows 4p..4p+3 per partition), and x loads are one 8KB/partition DMA per
16-tile mega. Software pipeline: fronts(m+2) att matmuls and chain(m+1)
(add/Prelu/Exp/den/recip/aw/awr4) are threaded into backs(m) at
dependency-ready points. PSUM start_tensor_calc marks a whole 2KB zero
region pending: exactly one start per mega per bank.

Sharding: pure data-parallel over batch: 512 batch rows per core,
weights replicated. fp16 I/O; host packs/upcasts.
"""

import os
from contextlib import ExitStack

import numpy as np

import concourse.bacc as bacc
import concourse.bass as bass
import concourse.tile as tile
from concourse import mybir
from concourse.bass_utils import run_bass_kernel_spmd

B, N, F = 4096, 64, 128
H, D = 8, 64
HD = H * D  # 512
NCORES = 8
BSHARD = B // NCORES  # 512
ROWS = BSHARD * N  # 32768
PAIR_ROWS = 512  # rows per pair (4 chunks of 128)
NPAIRS = ROWS // PAIR_ROWS  # 64
MPAIRS = 8  # pairs per mega
NMEGA = NPAIRS // MPAIRS  # 8
# half-pair slots (0..15) on pipeline A (DVE stt); rest pipeline B
A_SLOTS = (1, 4, 7, 10, 13)

f32 = mybir.dt.float32
f16 = mybir.dt.float16

LAST_RESULT = None  # test harness reads exec_time_ns / trace from here


def build_nc(nmega: int = NMEGA) -> bass.Bass:
    nc = bacc.Bacc("TRN2", target_bir_lowering=False, debug=False)
    rows = nmega * MPAIRS * PAIR_ROWS

    xt = nc.declare_dram_parameter("xt", [nmega, F, MPAIRS * PAIR_ROWS], f16,
                                   isOutput=False)
    wlin_d = nc.declare_dram_parameter("wlin", [F, HD], f16, isOutput=False)
    wc2_d = nc.declare_dram_parameter("wc2", [F, 8], f16, isOutput=False)
    bones_d = nc.declare_dram_parameter("bones", [128, 128], f16, isOutput=False)
    attm_d = nc.declare_dram_parameter("attm", [nmega, 128, 256], f16,
                                       isOutput=False)
    out = nc.declare_dram_parameter("out", [rows, HD], f16, isOutput=True)

    mult = mybir.AluOpType.mult
    mmax = mybir.AluOpType.max
    madd = mybir.AluOpType.add

    with tile.TileContext(nc) as tc, ExitStack() as ctx:
        consts = ctx.enter_context(tc.tile_pool(name="consts", bufs=1))
        xin = ctx.enter_context(tc.tile_pool(name="xin", bufs=6))
        amp = ctx.enter_context(tc.tile_pool(name="amp", bufs=3))
        rlinp = ctx.enter_context(tc.tile_pool(name="rlinp", bufs=4))
        outp = ctx.enter_context(tc.tile_pool(name="outp", bufs=4))
        plin = ctx.enter_context(tc.tile_pool(name="plin", bufs=3, space="PSUM"))
        pchain = ctx.enter_context(tc.tile_pool(name="pchain", bufs=1, space="PSUM"))

        # wc2 first: fronts(0) needs it; wlin/bones DMAs issue after loads(0)
        wc2_sb = consts.tile([F, 8], f16)
        nc.sync.dma_start(out=wc2_sb, in_=wc2_d[:])
        wlin_sb = consts.tile([F, HD], f16)
        bones_sb = consts.tile([128, 128], f16)
        alpha_sb = consts.tile([128, 1], f32)
        nc.vector.memset(alpha_sb, 0.2)
        ebias_sb = consts.tile([128, 1], f32)
        nc.vector.memset(ebias_sb, -4.0)

        # per-mega chain slabs, ping-pong x2. free-dim order is (j, k, h):
        # j pair 0..7 (stride 32), k chunk (stride 8), h head (stride 1)
        def mk_slabs(i):
            asum = consts.tile([128, 256], f16, tag=f"asum{i}")
            lk = consts.tile([128, 256], f16, tag=f"lk{i}")
            ew = consts.tile([128, 256], f16, tag=f"ew{i}")
            den4 = consts.tile([128, MPAIRS, H, 1], f32, tag=f"den4{i}")
            rden = consts.tile([128, MPAIRS, H, 1], f16, tag=f"rden{i}")
            aw = consts.tile([128, 256], f16, tag=f"aw{i}")
            awr4 = consts.tile([128, 256, 4], f16, tag=f"awr4{i}")
            return {"asum": asum, "lk": lk, "ew": ew, "den4": den4,
                    "rden": rden, "aw": aw, "awr4": awr4}

        slabs = [mk_slabs(0), mk_slabs(1)]
        # att accumulation + den, ping-pong PSUM: [:, 0, :] att, [:, 1, :] den
        att_ps = [pchain.tile([128, 2, 256], f32, tag=f"attps{i}",
                              name=f"attps{i}")
                  for i in range(2)]

        def loads(m):
            # two half-mega x DMAs so fronts can start after the first lands
            xa = xin.tile([F, 4, 4, 128], f16, tag="xa", name="xa")
            nc.sync.dma_start(
                out=xa.rearrange("f j k p -> f (j k p)"),
                in_=xt[m][:, 0:4 * PAIR_ROWS])
            xb = xin.tile([F, 4, 4, 128], f16, tag="xb", name="xb")
            nc.sync.dma_start(
                out=xb.rearrange("f j k p -> f (j k p)"),
                in_=xt[m][:, 4 * PAIR_ROWS:])
            am_sb = amp.tile([128, 256], f16, tag="am")
            nc.sync.dma_start(out=am_sb, in_=attm_d[m])

            def xv(j, k):
                return (xa if j < 4 else xb)[:, j % 4, k, :]

            return {"xv": xv, "am": am_sb}

        def fronts_pair(m, j, ld):
            # att[p, (j k h)] accumulates in PSUM. start_tensor_calc marks the
            # whole 2KB zero region pending: issue it ONLY on the mega's first
            # matmul; afterwards first touch of each address overwrites.
            av = att_ps[m % 2][:, 0, :].rearrange(
                "p (j k h) -> p j k h", j=MPAIRS, k=4, h=H)
            for k in range(4):
                nc.tensor.matmul(
                    av[:, j, k, :], ld["xv"](j, k), wc2_sb,
                    start=(j == 0 and k == 0),
                    stop=(j == MPAIRS - 1 and k == 3),
                    skip_group_check=True)

        def chain_add(m, ld):
            s = slabs[m % 2]
            nc.vector.tensor_tensor(out=s["asum"], in0=att_ps[m % 2][:, 0, :],
                                    in1=ld["am"], op=madd)

        def chain_act(m):
            s = slabs[m % 2]
            nc.scalar.activation(
                out=s["lk"], in_=s["asum"],
                func=mybir.ActivationFunctionType.Prelu, alpha=alpha_sb)
            nc.scalar.activation(
                out=s["ew"], in_=s["lk"],
                func=mybir.ActivationFunctionType.Exp, bias=ebias_sb)

        def chain_den(m):
            # den region is pending-zero from the mega's fronts start flag:
            # start=False means this first write overwrites, not accumulates
            s = slabs[m % 2]
            nc.tensor.matmul(att_ps[m % 2][:, 1, :], bones_sb, s["ew"],
                             start=False, stop=True, skip_group_check=True)

        def chain_dve(m):
            s = slabs[m % 2]
            dv = att_ps[m % 2][:, 1, :].rearrange(
                "p (j k h) -> p j h k", j=MPAIRS, k=4, h=H)
            nc.vector.tensor_reduce(out=s["den4"], in_=dv,
                                    axis=mybir.AxisListType.X, op=madd)
            with nc.allow_low_precision(reason="aw weights are f16 anyway"):
                nc.vector.reciprocal(s["rden"], s["den4"])

        def chain_awt(m):
            s = slabs[m % 2]
            awv = s["aw"].rearrange("p (j k h) -> p j k h", j=MPAIRS, k=4, h=H)
            ewv = s["ew"].rearrange("p (j k h) -> p j k h", j=MPAIRS, k=4, h=H)
            rdv = s["rden"].rearrange("p j h one -> p j (h one)")
            for k in range(4):
                nc.gpsimd.tensor_tensor(
                    out=awv[:, :, k, :], in0=ewv[:, :, k, :], in1=rdv, op=mult)

        def chain_awr4(m):
            s = slabs[m % 2]
            nc.scalar.copy(out=s["awr4"],
                           in_=s["aw"].to_broadcast([128, 256, 4]))

        def backs_half(m, j, u, ld, state):
            s = slabs[m % 2]
            slot = j * 2 + u
            if u == 0:
                state["o2"] = outp.tile([128, 4, HD], f16, tag="o2",
                                        name="o2")
            o2 = state["o2"]
            lin_ps = plin.tile([128, 2, HD], f32, tag="lin")
            for c in (0, 1):
                nc.tensor.matmul(lin_ps[:, c, :], ld["xv"](j, 2 * u + c),
                                 wlin_sb, start=True, stop=True)
            bb_pair = (2 * j not in A_SLOTS) and (2 * j + 1 not in A_SLOTS)
            if slot in A_SLOTS:
                aw16 = s["aw"][:, j * 32 + 16 * u:j * 32 + 16 * u + 16]
                nc.vector.scalar_tensor_tensor(
                    out=o2[:, 2 * u:2 * u + 2, :].rearrange(
                        "p k (h d) -> p (k h) d", h=H),
                    in0=lin_ps.rearrange("p k (h d) -> p (k h) d", h=H),
                    scalar=0.0,
                    in1=aw16.to_broadcast([128, 2 * H, D]),
                    op0=mmax, op1=mult)
            elif bb_pair:
                # both halves B: one merged [128, 2048] tt for the whole pair
                if u == 0:
                    state["rlbb"] = rlinp.tile([128, 4, HD], f16, tag="rlbb",
                                               name="rlbb")
                rlin4 = state["rlbb"]
                nc.scalar.activation(
                    out=rlin4[:, 2 * u:2 * u + 2, :].rearrange(
                        "p a b -> p (a b)"),
                    in_=lin_ps.rearrange("p a b -> p (a b)"),
                    func=mybir.ActivationFunctionType.Relu)
                if u == 1:
                    state.setdefault("pend", []).append(
                        ("BB", m, j, 0, rlin4, o2))
            else:
                rlin = rlinp.tile([128, 2, HD], f16, tag="rlin")
                nc.scalar.activation(
                    out=rlin.rearrange("p a b -> p (a b)"),
                    in_=lin_ps.rearrange("p a b -> p (a b)"),
                    func=mybir.ActivationFunctionType.Relu)
                # defer the DVE tt one half-pair so ACT has lead time
                state.setdefault("pend", []).append(("B", m, j, u, rlin, o2))
            if u == 1:
                state.setdefault("dma", []).append((j, o2))

        def flush_tts(state, keep=1):
            s_pend = state.get("pend", [])
            while len(s_pend) > keep:
                kind, m, j, u, rlin, o2 = s_pend.pop(0)
                s = slabs[m % 2]
                a4v = s["awr4"].rearrange(
                    "p (j kh) r -> p j kh r", j=MPAIRS, kh=32)
                if kind == "BB":
                    in1 = a4v[:, j, :, :].rearrange(
                        "p kh (one r) -> p kh one r", one=1
                    ).broadcast_to([128, 32, 16, 4])
                    nc.vector.tensor_tensor(
                        out=o2.rearrange(
                            "p k (h da db) -> p (k h) da db", h=H, da=16),
                        in0=rlin.rearrange(
                            "p k (h da db) -> p (k h) da db", h=H, da=16),
                        in1=in1, op=mult)
                else:
                    in1 = a4v[:, j, 16 * u:16 * u + 16, :].rearrange(
                        "p kh (one r) -> p kh one r", one=1
                    ).broadcast_to([128, 16, 16, 4])
                    nc.vector.tensor_tensor(
                        out=o2[:, 2 * u:2 * u + 2, :].rearrange(
                            "p k (h da db) -> p (k h) da db", h=H, da=16),
                        in0=rlin.rearrange(
                            "p k (h da db) -> p (k h) da db", h=H, da=16),
                        in1=in1, op=mult)

        def flush_dma(m, state, keep=1):
            dmas = state.get("dma", [])
            while len(dmas) > keep:
                j, o2 = dmas.pop(0)
                base = (m * MPAIRS + j) * PAIR_ROWS
                out_view = out[base:base + PAIR_ROWS, :].rearrange(
                    "(p four) hd -> p (four hd)", four=4)
                nc.sync.dma_start(
                    out=out_view, in_=o2.rearrange("p k hd -> p (k hd)"))

        # ---- software pipeline ----
        ld = {0: loads(0)}
        nc.sync.dma_start(out=wlin_sb, in_=wlin_d[:])
        nc.sync.dma_start(out=bones_sb, in_=bones_d[:])
        for j in range(MPAIRS):
            fronts_pair(0, j, ld[0])
        chain_add(0, ld[0])
        chain_act(0)
        chain_den(0)
        chain_dve(0)
        chain_awt(0)
        chain_awr4(0)
        if nmega > 1:
            ld[1] = loads(1)
            for j in range(MPAIRS):
                fronts_pair(1, j, ld[1])

        for m in range(nmega):
            state = {}
            for j in range(MPAIRS):
                for u in (0, 1):
                    backs_half(m, j, u, ld[m], state)
                    flush_tts(state, keep=1)
                flush_dma(m, state, keep=1)
                if j == 0 and m + 2 < nmega:
                    ld[m + 2] = loads(m + 2)
                if m + 1 < nmega:
                    if j == 1:
                        chain_add(m + 1, ld[m + 1])
                    elif j == 2:
                        chain_act(m + 1)
                    elif j == 3:
                        chain_den(m + 1)
                    elif j == 4:
                        chain_dve(m + 1)
                    elif j == 5:
                        chain_awt(m + 1)
                    elif j == 6:
                        chain_awr4(m + 1)
                if m + 2 < nmega and j >= 4:
                    fronts_pair(m + 2, 2 * (j - 4), ld[m + 2])
                    fronts_pair(m + 2, 2 * (j - 4) + 1, ld[m + 2])
            flush_tts(state, keep=0)
            flush_dma(m, state, keep=0)
            ld.pop(m, None)

    nc.compile()
    return nc


def _host_weights(W_lin, W_att):
    W_lin64 = W_lin.astype(np.float64)
    wc2 = (W_lin64 @ W_att[HD:].astype(np.float64)).astype(np.float16)
    wc1 = (W_lin64 @ W_att[:HD].astype(np.float64)).astype(np.float32)
    bones = np.zeros((128, 128), dtype=np.float16)
    for b in range(8):
        bones[16 * b:16 * b + 16, 16 * b:16 * b + 16] = 1.0
    return W_lin.astype(np.float16), wc2, wc1, bones


def _core_inputs(x_shard, mask_shard, wlin, wc2, wc1):
    nb = x_shard.shape[0]
    rows = nb * N
    nmega = rows // (MPAIRS * PAIR_ROWS)
    xsh = x_shard.reshape(rows, F)
    # x_pack[m, f, (j, k, p)] = x[row = m*4096 + j*512 + 4p + k, f]
    xp = (xsh.reshape(nmega, MPAIRS, 128, 4, F)
          .transpose(0, 4, 1, 3, 2)
          .reshape(nmega, F, MPAIRS * PAIR_ROWS)
          .astype(np.float16))
    # attm[m, p, (j, k, h)] = attsrc[batch(j, p//16), h] + {0, -6e4}[mask]
    attsrc = x_shard[:, 0, :].astype(np.float32) @ wc1  # [nb, H]
    a1 = attsrc.reshape(nmega, MPAIRS, 8, H)
    a2 = np.repeat(a1, 16, axis=2)                      # m, j, p, h
    a3 = a2.transpose(0, 2, 1, 3)                       # m, p, j, h
    a4 = np.broadcast_to(a3[:, :, :, None, :], (nmega, 128, MPAIRS, 4, H))
    mb = np.where(mask_shard.reshape(rows) != 0, 0.0, -60000.0).astype(
        np.float32)
    mv = mb.reshape(nmega, MPAIRS, 128, 4).transpose(0, 2, 1, 3)  # m,p,j,k
    attm = (a4 + mv[..., None]).astype(np.float16).reshape(nmega, 128, 256)
    return {
        "xt": np.ascontiguousarray(xp),
        "wlin": wlin,
        "wc2": wc2,
        "bones": np.zeros((0,)),  # replaced below
        "attm": np.ascontiguousarray(attm),
    }


def kernel(x, W_lin, W_att, mask):
    global LAST_RESULT
    x = np.asarray(x, dtype=np.float32)
    W_lin = np.asarray(W_lin, dtype=np.float32)
    W_att = np.asarray(W_att, dtype=np.float32)
    mask = np.asarray(mask)

    wlin, wc2, wc1, bones = _host_weights(W_lin, W_att)
    in_maps = []
    for c in range(NCORES):
        im = _core_inputs(
            x[c * BSHARD:(c + 1) * BSHARD],
            mask[c * BSHARD:(c + 1) * BSHARD],
            wlin, wc2, wc1,
        )
        im["bones"] = bones
        in_maps.append(im)

    nc = build_nc(NMEGA)
    trace = os.environ.get("KERNEL_TRACE", "0") == "1"
    tmpdir = os.environ.get("KERNEL_TRACE_DIR") or None
    res = run_bass_kernel_spmd(
        nc, in_maps, list(range(NCORES)), trace=trace, tmpdir=tmpdir
    )
    LAST_RESULT = res
    return np.concatenate(
        [
            res.results[c]["out"].astype(np.float32).reshape(BSHARD, N, HD)
            for c in range(NCORES)
        ],
        axis=0,
    )


# revision 27
# speedup vs baseline: 1.2271x; 1.0192x over previous
"""Trainium2 Bass kernel for nn_AttentionAggregator (GNN message passing).

Math (per batch row b, N=64 neighbors, F=128 in-feat, H=8 heads, D=64):
    lin  = x @ W_lin                                      [B, N, 512]
    att  = lin[:,0,:] @ W_att[:512] + lin @ W_att[512:]   [B, N, 8]
    att  = LeakyReLU_0.2(att); masked softmax over N per (b, h)
    out  = relu(lin) * aw                                 [B, N, 512]

Design (v4, row-major attention): the original design computed attention
head-major and burned ~100us/core of PE on head-expander/transpose matmuls
plus 512-col triplet streams. v4 computes att ROW-major on the PE as one
tiny 8-col matmul per 128-row chunk (att[row, :] = x_chunk.T @ wc2, with
wc2 = W_lin @ W_att[512:]). The src term (x[b,0] @ wc1, 8 MFLOP = 0.02% of
model FLOPs) plus the {0,-6e4} mask bias are packed on the host into a
per-row f16 bias slab, added by one DVE op per mega before the LeakyReLU
-- both are legal pre-softmax because LeakyReLU(x - big) still exps to 0
and the src add commutes with nothing else in between.

Rows are host-permuted within each 512-row pair so row = 4*p + k (k =
chunk 0..3, p = partition); a batch element's 64 rows then live in a
16-partition block across 4 chunk-columns, so the softmax denominator is
one block-ones16 matmul (partition sum, broadcast for free) + a DVE
k-reduce + reciprocal. aw comes out row-major [p, (j k h)] -- no
transposes or expanders at all.

Final multiply per 256-row half-pair, two balanced paths:
  A (DVE):      stt out = max(lin_f32psum,0) * aw_bcast        (~1.45us)
  B (ACT+DVE):  ACT relu lin->rlin f16 sbuf                    (~1.3us)
                one DVE tt rlin * awr4 view                    (~0.65us)
awr4 is aw replicated 4x along a packed last dim (one ACT broadcast-copy
per mega): viewing d = 16*dA + dB with dA stride-0 and dB stride-1 keeps
enough 16-bit packing for the DVE 2x mode, unlike a plain stride-0
broadcast which drops DVE to 1x (measured).

The row permutation also makes each output-DMA descriptor 4KB contiguous
(rows 4p..4p+3 per partition), and x loads are one 8KB/partition DMA per
16-tile mega. Software pipeline: fronts(m+2) att matmuls and chain(m+1)
(add/Prelu/Exp/den/recip/aw/awr4) are threaded into backs(m) at
dependency-ready points. PSUM start_tensor_calc marks a whole 2KB zero
region pending: exactly one start per mega per bank.

Sharding: pure data-parallel over batch: 512 batch rows per core,
weights replicated. fp16 I/O; host packs/upcasts.
"""

import os
from contextlib import ExitStack

import numpy as np

import concourse.bacc as bacc
import concourse.bass as bass
import concourse.tile as tile
from concourse import mybir
from concourse.bass_utils import run_bass_kernel_spmd

B, N, F = 4096, 64, 128
H, D = 8, 64
HD = H * D  # 512
NCORES = 8
BSHARD = B // NCORES  # 512
ROWS = BSHARD * N  # 32768
PAIR_ROWS = 512  # rows per pair (4 chunks of 128)
NPAIRS = ROWS // PAIR_ROWS  # 64
MPAIRS = 8  # pairs per mega
NMEGA = NPAIRS // MPAIRS  # 8
# half-pair slots (0..15) on pipeline A (DVE stt); rest pipeline B
A_SLOTS = (1, 4, 7, 10, 13)

f32 = mybir.dt.float32
f16 = mybir.dt.float16

LAST_RESULT = None  # test harness reads exec_time_ns / trace from here


def build_nc(nmega: int = NMEGA) -> bass.Bass:
    nc = bacc.Bacc("TRN2", target_bir_lowering=False, debug=False)
    rows = nmega * MPAIRS * PAIR_ROWS

    xt = nc.declare_dram_parameter("xt", [nmega, F, MPAIRS * PAIR_ROWS], f16,
                                   isOutput=False)
    wlin_d = nc.declare_dram_parameter("wlin", [F, HD], f16, isOutput=False)
    wc2_d = nc.declare_dram_parameter("wc2", [F, 8], f16, isOutput=False)
    bones_d = nc.declare_dram_parameter("bones", [128, 128], f16, isOutput=False)
    attm_d = nc.declare_dram_parameter("attm", [nmega, 128, 256], f16,
                                       isOutput=False)
    out = nc.declare_dram_parameter("out", [rows, HD], f16, isOutput=True)

    mult = mybir.AluOpType.mult
    mmax = mybir.AluOpType.max
    madd = mybir.AluOpType.add

    with tile.TileContext(nc) as tc, ExitStack() as ctx:
        consts = ctx.enter_context(tc.tile_pool(name="consts", bufs=1))
        xin = ctx.enter_context(tc.tile_pool(name="xin", bufs=6))
        amp = ctx.enter_context(tc.tile_pool(name="amp", bufs=3))
        rlinp = ctx.enter_context(tc.tile_pool(name="rlinp", bufs=4))
        outp = ctx.enter_context(tc.tile_pool(name="outp", bufs=4))
        plin = ctx.enter_context(tc.tile_pool(name="plin", bufs=3, space="PSUM"))
        pchain = ctx.enter_context(tc.tile_pool(name="pchain", bufs=1, space="PSUM"))

        # wc2 first: fronts(0) needs it; wlin/bones DMAs issue after loads(0)
        wc2_sb = consts.tile([F, 8], f16)
        nc.sync.dma_start(out=wc2_sb, in_=wc2_d[:])
        wlin_sb = consts.tile([F, HD], f16)
        bones_sb = consts.tile([128, 128], f16)
        alpha_sb = consts.tile([128, 1], f32)
        nc.vector.memset(alpha_sb, 0.2)
        ebias_sb = consts.tile([128, 1], f32)
        nc.vector.memset(ebias_sb, -4.0)

        # per-mega chain slabs, ping-pong x2. free-dim order is (j, k, h):
        # j pair 0..7 (stride 32), k chunk (stride 8), h head (stride 1)
        def mk_slabs(i):
            asum = consts.tile([128, 256], f16, tag=f"asum{i}")
            lk = consts.tile([128, 256], f16, tag=f"lk{i}")
            ew = consts.tile([128, 256], f16, tag=f"ew{i}")
            den4 = consts.tile([128, MPAIRS, H, 1], f32, tag=f"den4{i}")
            rden = consts.tile([128, MPAIRS, H, 1], f16, tag=f"rden{i}")
            aw = consts.tile([128, 256], f16, tag=f"aw{i}")
            awr4 = consts.tile([128, 256, 4], f16, tag=f"awr4{i}")
            return {"asum": asum, "lk": lk, "ew": ew, "den4": den4,
                    "rden": rden, "aw": aw, "awr4": awr4}

        slabs = [mk_slabs(0), mk_slabs(1)]
        # att accumulation + den, ping-pong PSUM: [:, 0, :] att, [:, 1, :] den
        att_ps = [pchain.tile([128, 2, 256], f32, tag=f"attps{i}",
                              name=f"attps{i}")
                  for i in range(2)]

        def loads(m):
            # two half-mega x DMAs so fronts can start after the first lands
            xa = xin.tile([F, 4, 4, 128], f16, tag="xa", name="xa")
            nc.sync.dma_start(
                out=xa.rearrange("f j k p -> f (j k p)"),
                in_=xt[m][:, 0:4 * PAIR_ROWS])
            xb = xin.tile([F, 4, 4, 128], f16, tag="xb", name="xb")
            nc.sync.dma_start(
                out=xb.rearrange("f j k p -> f (j k p)"),
                in_=xt[m][:, 4 * PAIR_ROWS:])
            am_sb = amp.tile([128, 256], f16, tag="am")
            nc.sync.dma_start(out=am_sb, in_=attm_d[m])

            def xv(j, k):
                return (xa if j < 4 else xb)[:, j % 4, k, :]

            return {"xv": xv, "am": am_sb}

        def fronts_pair(m, j, ld):
            # att[p, (j k h)] accumulates in PSUM. start_tensor_calc marks the
            # whole 2KB zero region pending: issue it ONLY on the mega's first
            # matmul; afterwards first touch of each address overwrites.
            av = att_ps[m % 2][:, 0, :].rearrange(
                "p (j k h) -> p j k h", j=MPAIRS, k=4, h=H)
            for k in range(4):
                nc.tensor.matmul(
                    av[:, j, k, :], ld["xv"](j, k), wc2_sb,
                    start=(j == 0 and k == 0),
                    stop=(j == MPAIRS - 1 and k == 3),
                    skip_group_check=True)

        def chain_add(m, ld):
            s = slabs[m % 2]
            nc.vector.tensor_tensor(out=s["asum"], in0=att_ps[m % 2][:, 0, :],
                                    in1=ld["am"], op=madd)

        def chain_act(m):
            s = slabs[m % 2]
            nc.scalar.activation(
                out=s["lk"], in_=s["asum"],
                func=mybir.ActivationFunctionType.Prelu, alpha=alpha_sb)
            nc.scalar.activation(
                out=s["ew"], in_=s["lk"],
                func=mybir.ActivationFunctionType.Exp, bias=ebias_sb)

        def chain_den(m):
            # den region is pending-zero from the mega's fronts start flag:
            # start=False means this first write overwrites, not accumulates
            s = slabs[m % 2]
            nc.tensor.matmul(att_ps[m % 2][:, 1, :], bones_sb, s["ew"],
                             start=False, stop=True, skip_group_check=True)

        def chain_dve(m):
            s = slabs[m % 2]
            dv = att_ps[m % 2][:, 1, :].rearrange(
                "p (j k h) -> p j h k", j=MPAIRS, k=4, h=H)
            nc.vector.tensor_reduce(out=s["den4"], in_=dv,
                                    axis=mybir.AxisListType.X, op=madd)
            with nc.allow_low_precision(reason="aw weights are f16 anyway"):
                nc.vector.reciprocal(s["rden"], s["den4"])

        def chain_awt(m):
            s = slabs[m % 2]
            awv = s["aw"].rearrange("p (j k h) -> p j k h", j=MPAIRS, k=4, h=H)
            ewv = s["ew"].rearrange("p (j k h) -> p j k h", j=MPAIRS, k=4, h=H)
            rdv = s["rden"].rearrange("p j h one -> p j (h one)")
            for k in range(4):
                nc.gpsimd.tensor_tensor(
                    out=awv[:, :, k, :], in0=ewv[:, :, k, :], in1=rdv, op=mult)

        def chain_awr4(m):
            s = slabs[m % 2]
            nc.scalar.copy(out=s["awr4"],
                           in_=s["aw"].to_broadcast([128, 256, 4]))

        def backs_half(m, j, u, ld, state):
            s = slabs[m % 2]
            slot = j * 2 + u
            if u == 0:
                state["o2"] = outp.tile([128, 4, HD], f16, tag="o2",
                                        name="o2")
            o2 = state["o2"]
            lin_ps = plin.tile([128, 2, HD], f32, tag="lin")
            for c in (0, 1):
                nc.tensor.matmul(lin_ps[:, c, :], ld["xv"](j, 2 * u + c),
                                 wlin_sb, start=True, stop=True)
            bb_pair = (2 * j not in A_SLOTS) and (2 * j + 1 not in A_SLOTS)
            if slot in A_SLOTS:
                aw16 = s["aw"][:, j * 32 + 16 * u:j * 32 + 16 * u + 16]
                nc.vector.scalar_tensor_tensor(
                    out=o2[:, 2 * u:2 * u + 2, :].rearrange(
                        "p k (h d) -> p (k h) d", h=H),
                    in0=lin_ps.rearrange("p k (h d) -> p (k h) d", h=H),
                    scalar=0.0,
                    in1=aw16.to_broadcast([128, 2 * H, D]),
                    op0=mmax, op1=mult)
            elif bb_pair:
                # both halves B: one merged [128, 2048] tt for the whole pair
                if u == 0:
                    state["rlbb"] = rlinp.tile([128, 4, HD], f16, tag="rlbb",
                                               name="rlbb")
                rlin4 = state["rlbb"]
                nc.scalar.activation(
                    out=rlin4[:, 2 * u:2 * u + 2, :].rearrange(
                        "p a b -> p (a b)"),
                    in_=lin_ps.rearrange("p a b -> p (a b)"),
                    func=mybir.ActivationFunctionType.Relu)
                if u == 1:
                    state.setdefault("pend", []).append(
                        ("BB", m, j, 0, rlin4, o2))
            else:
                rlin = rlinp.tile([128, 2, HD], f16, tag="rlin")
                nc.scalar.activation(
                    out=rlin.rearrange("p a b -> p (a b)"),
                    in_=lin_ps.rearrange("p a b -> p (a b)"),
                    func=mybir.ActivationFunctionType.Relu)
                # defer the DVE tt one half-pair so ACT has lead time
                state.setdefault("pend", []).append(("B", m, j, u, rlin, o2))
            if u == 1:
                state.setdefault("dma", []).append((j, o2))

        def flush_tts(state, keep=1):
            s_pend = state.get("pend", [])
            while len(s_pend) > keep:
                kind, m, j, u, rlin, o2 = s_pend.pop(0)
                s = slabs[m % 2]
                a4v = s["awr4"].rearrange(
                    "p (j kh) r -> p j kh r", j=MPAIRS, kh=32)
                if kind == "BB":
                    in1 = a4v[:, j, :, :].rearrange(
                        "p kh (one r) -> p kh one r", one=1
                    ).broadcast_to([128, 32, 16, 4])
                    nc.vector.tensor_tensor(
                        out=o2.rearrange(
                            "p k (h da db) -> p (k h) da db", h=H, da=16),
                        in0=rlin.rearrange(
                            "p k (h da db) -> p (k h) da db", h=H, da=16),
                        in1=in1, op=mult)
                else:
                    in1 = a4v[:, j, 16 * u:16 * u + 16, :].rearrange(
                        "p kh (one r) -> p kh one r", one=1
                    ).broadcast_to([128, 16, 16, 4])
                    eng = nc.gpsimd if (j * 2 + u) == 14 else nc.vector
                    eng.tensor_tensor(
                        out=o2[:, 2 * u:2 * u + 2, :].rearrange(
                            "p k (h da db) -> p (k h) da db", h=H, da=16),
                        in0=rlin.rearrange(
                            "p k (h da db) -> p (k h) da db", h=H, da=16),
                        in1=in1, op=mult)

        def flush_dma(m, state, keep=1):
            dmas = state.get("dma", [])
            while len(dmas) > keep:
                j, o2 = dmas.pop(0)
                base = (m * MPAIRS + j) * PAIR_ROWS
                out_view = out[base:base + PAIR_ROWS, :].rearrange(
                    "(p four) hd -> p (four hd)", four=4)
                nc.sync.dma_start(
                    out=out_view, in_=o2.rearrange("p k hd -> p (k hd)"))

        # ---- software pipeline ----
        ld = {0: loads(0)}
        nc.sync.dma_start(out=wlin_sb, in_=wlin_d[:])
        nc.sync.dma_start(out=bones_sb, in_=bones_d[:])
        for j in range(MPAIRS):
            fronts_pair(0, j, ld[0])
        chain_add(0, ld[0])
        chain_act(0)
        chain_den(0)
        chain_dve(0)
        chain_awt(0)
        chain_awr4(0)
        if nmega > 1:
            ld[1] = loads(1)
            for j in range(MPAIRS):
                fronts_pair(1, j, ld[1])

        for m in range(nmega):
            state = {}
            for j in range(MPAIRS):
                for u in (0, 1):
                    backs_half(m, j, u, ld[m], state)
                    flush_tts(state, keep=1)
                flush_dma(m, state, keep=1)
                if j == 0 and m + 2 < nmega:
                    ld[m + 2] = loads(m + 2)
                if m + 1 < nmega:
                    if j == 0:
                        chain_add(m + 1, ld[m + 1])
                    elif j == 1:
                        chain_act(m + 1)
                    elif j == 2:
                        chain_den(m + 1)
                    elif j == 3:
                        chain_dve(m + 1)
                    elif j == 4:
                        chain_awt(m + 1)
                    elif j == 5:
                        chain_awr4(m + 1)
                if m + 2 < nmega and j >= 4:
                    fronts_pair(m + 2, 2 * (j - 4), ld[m + 2])
                    fronts_pair(m + 2, 2 * (j - 4) + 1, ld[m + 2])
            flush_tts(state, keep=0)
            flush_dma(m, state, keep=0)
            ld.pop(m, None)

    nc.compile()
    return nc


def _host_weights(W_lin, W_att):
    W_lin64 = W_lin.astype(np.float64)
    wc2 = (W_lin64 @ W_att[HD:].astype(np.float64)).astype(np.float16)
    wc1 = (W_lin64 @ W_att[:HD].astype(np.float64)).astype(np.float32)
    bones = np.zeros((128, 128), dtype=np.float16)
    for b in range(8):
        bones[16 * b:16 * b + 16, 16 * b:16 * b + 16] = 1.0
    return W_lin.astype(np.float16), wc2, wc1, bones


def _core_inputs(x_shard, mask_shard, wlin, wc2, wc1):
    nb = x_shard.shape[0]
    rows = nb * N
    nmega = rows // (MPAIRS * PAIR_ROWS)
    xsh = x_shard.reshape(rows, F)
    # x_pack[m, f, (j, k, p)] = x[row = m*4096 + j*512 + 4p + k, f]
    xp = (xsh.reshape(nmega, MPAIRS, 128, 4, F)
          .transpose(0, 4, 1, 3, 2)
          .reshape(nmega, F, MPAIRS * PAIR_ROWS)
          .astype(np.float16))
    # attm[m, p, (j, k, h)] = attsrc[batch(j, p//16), h] + {0, -6e4}[mask]
    attsrc = x_shard[:, 0, :].astype(np.float32) @ wc1  # [nb, H]
    a1 = attsrc.reshape(nmega, MPAIRS, 8, H)
    a2 = np.repeat(a1, 16, axis=2)                      # m, j, p, h
    a3 = a2.transpose(0, 2, 1, 3)                       # m, p, j, h
    a4 = np.broadcast_to(a3[:, :, :, None, :], (nmega, 128, MPAIRS, 4, H))
    mb = np.where(mask_shard.reshape(rows) != 0, 0.0, -60000.0).astype(
        np.float32)
    mv = mb.reshape(nmega, MPAIRS, 128, 4).transpose(0, 2, 1, 3)  # m,p,j,k
    attm = (a4 + mv[..., None]).astype(np.float16).reshape(nmega, 128, 256)
    return {
        "xt": np.ascontiguousarray(xp),
        "wlin": wlin,
        "wc2": wc2,
        "bones": np.zeros((0,)),  # replaced below
        "attm": np.ascontiguousarray(attm),
    }


def kernel(x, W_lin, W_att, mask):
    global LAST_RESULT
    x = np.asarray(x, dtype=np.float32)
    W_lin = np.asarray(W_lin, dtype=np.float32)
    W_att = np.asarray(W_att, dtype=np.float32)
    mask = np.asarray(mask)

    wlin, wc2, wc1, bones = _host_weights(W_lin, W_att)
    in_maps = []
    for c in range(NCORES):
        im = _core_inputs(
            x[c * BSHARD:(c + 1) * BSHARD],
            mask[c * BSHARD:(c + 1) * BSHARD],
            wlin, wc2, wc1,
        )
        im["bones"] = bones
        in_maps.append(im)

    nc = build_nc(NMEGA)
    trace = os.environ.get("KERNEL_TRACE", "0") == "1"
    tmpdir = os.environ.get("KERNEL_TRACE_DIR") or None
    res = run_bass_kernel_spmd(
        nc, in_maps, list(range(NCORES)), trace=trace, tmpdir=tmpdir
    )
    LAST_RESULT = res
    return np.concatenate(
        [
            res.results[c]["out"].astype(np.float32).reshape(BSHARD, N, HD)
            for c in range(NCORES)
        ],
        axis=0,
    )
